# revision 2
# baseline (speedup 1.0000x reference)
"""MHA + residual + LayerNorm, 8-core SPMD Trainium2 kernel — fp8-DR rev.

Reference (B=4, S=2048, H=1024, 16 heads x 64):
    q/k/v = hs @ W{q,k,v}.T + b     probs = softmax(qk^T/8)
    ctx = probs@v;  out = LN(ctx@Wo.T + bo + hs) * gamma + beta

Sharding: 8 shards = (batch, seq half); each core owns 1024 query rows,
computes K/V over the full 2048 keys of its batch (no collectives).

Numerics:
    weights   host-scaled x16 to fp8e4 (w ~ N(0,.5) in fp8 normal range);
              all four projections run fp8 DoubleRow (2 k-tiles/matmul, 2x PE)
    hsT8      [128,8,2048] fp8 transposed hidden states (PE transpose+DVE cast)
    kT/qT     bf16 [d,s] per head-pair; evict scale 1/16 (k), 1.4427/16 (q)
              so scores PSUM = 11.5416*s  (s = q.k/8)
    probs     bf16 = exp(s)/8 on ACT (scale=1/11.5416, bias=-ln8); a few
              key-tiles per head optionally on DVE via Schraudolph int16:
              bits = round(psum*16 + 15872+c) -> bitcast bf16 (~3% rel err)
    vS        bf16 [128,16*65] per key-tile, ones col -> softmax sums come out
              of the bf16 ctx matmul row 64
    cT8       fp8 = 4*ctx/sums via gathered reciprocal: sums rows -> DRAM ->
              [8,128] -> x4 -> reciprocal -> DRAM -> [64,1024] broadcast
    out proj  fp8 DR: PSUM = 64*attn; residual hs scaled x64 (LN scale-inv)
"""

import numpy as np

import concourse.bass as bass
import concourse.mybir as mybir
import concourse.tile as tile
from concourse import bacc
from concourse.masks import make_identity
from concourse.bass_utils import run_bass_kernel_spmd

F32 = mybir.dt.float32
BF16 = mybir.dt.bfloat16
FP8 = mybir.dt.float8e4
I16 = mybir.dt.int16
AF = mybir.ActivationFunctionType
OP = mybir.AluOpType
DR = mybir.MatmulPerfMode.DoubleRow

B, S, H = 4, 2048, 1024
NH, HD = 16, 64
SH = S // 2
N_CORES = 8
EPS = 1e-12

HT = H // 128        # 8 contraction tiles
ST = S // 128        # 16 key tiles
QB = SH // 512       # 2 q chunks
HP = NH // 2         # 8 head-pair tiles

A_SCHR = 11.541560327111707     # psum = A*s with A = 8*log2(e)
C16 = 15872.0 - 7.0             # bf16 Schraudolph: bits = 16*psum + C16
LN8 = 2.0794415416798357
W_SCALE = 16.0

N_DVE_KT = 0                    # key-tiles per head exp'd on DVE (of 16)

_CACHED_NC = {}


def _emit(tc, ln_id, zb):
    nc = tc.nc
    hs_q = nc.dram_tensor("hs_q", [SH, H], F32, kind="ExternalInput").ap()
    hs_o = nc.dram_tensor("hs_o", [SH, H], F32, kind="ExternalInput").ap()
    wq8_d = nc.dram_tensor("wq8", [H, H], FP8, kind="ExternalInput").ap()
    wk8_d = nc.dram_tensor("wk8", [H, H], FP8, kind="ExternalInput").ap()
    wv8_d = nc.dram_tensor("wv8", [H, H], FP8, kind="ExternalInput").ap()
    wo8_d = nc.dram_tensor("wo8", [H, H], FP8, kind="ExternalInput").ap()
    bq_d = nc.dram_tensor("bq", [H], F32, kind="ExternalInput").ap()   # x1.4427
    bk_d = nc.dram_tensor("bk", [H], F32, kind="ExternalInput").ap()
    bv_d = nc.dram_tensor("bv", [H], BF16, kind="ExternalInput").ap()
    bo_d = nc.dram_tensor("bo", [H], F32, kind="ExternalInput").ap()   # x64
    gam_d = nc.dram_tensor("ln_gamma", [H], F32, kind="ExternalInput").ap()
    bet_d = nc.dram_tensor("ln_beta", [H], F32, kind="ExternalInput").ap()
    out_d = nc.dram_tensor("out", [SH, H], F32, kind="ExternalOutput").ap()

    # ---------------- persistent tiles ----------------
    persist = tc.alloc_tile_pool(name="persist", bufs=1)
    hsT8 = persist.tile([128, HT, S], FP8, name="hsT8")
    kT = [persist.tile([128, S], BF16, name=f"kT{i}") for i in range(HP)]
    qT = [persist.tile([128, SH], BF16, name=f"qT{i}") for i in range(HP)]
    vS = [persist.tile([128, NH * (HD + 1)], BF16, name=f"vS{i}")
          for i in range(ST)]
    cT8 = persist.tile([128, HT, SH], FP8, name="cT8")

    const_p = tc.alloc_tile_pool(name="const", bufs=1)
    eps_t = const_p.tile([128, 1], F32, name="eps_t")
    nc.vector.memset(eps_t, EPS)
    nl8_t = const_p.tile([128, 1], F32, name="nl8_t")
    nc.vector.memset(nl8_t, -LN8)
    bqc = const_p.tile([128, HT], F32, name="bqc")
    nc.sync.dma_start(out=bqc, in_=bq_d.rearrange("(j p) -> p j", p=128))
    bkc = const_p.tile([128, HT], F32, name="bkc")
    nc.sync.dma_start(out=bkc, in_=bk_d.rearrange("(j p) -> p j", p=128))
    if not zb:
        bvb = const_p.tile([128, H], BF16, name="bvb")
        nc.sync.dma_start(out=bvb,
                          in_=bv_d.rearrange("(o n) -> o n", o=1).partition_broadcast(128))
    ident = const_p.tile([128, 128], BF16, name="ident")
    make_identity(nc, ident)

    # ---------------- streaming pools ----------------
    mm_ps = tc.alloc_tile_pool(name="mmps", bufs=2, space="PSUM")
    sc_ps = tc.alloc_tile_pool(name="scps", bufs=2, space="PSUM")
    cx_ps = tc.alloc_tile_pool(name="cxps", bufs=1, space="PSUM")
    dram_pool = tc.alloc_tile_pool(name="drampool", bufs=1, space="DRAM")
    nrm_pool = tc.alloc_tile_pool(name="nrmpool", bufs=2)
    p_pool = tc.alloc_tile_pool(name="ppool", bufs=4)
    wkq_pool = tc.alloc_tile_pool(name="wkqpool", bufs=1)
    tr_pool = tc.alloc_tile_pool(name="trpool", bufs=5)

    def load_w8(pool, dram, nm):
        w = pool.tile([128, HT, H], FP8, name=nm)
        nc.sync.dma_start(out=w, in_=dram.rearrange("(t p) n -> p t n", p=128))
        return w

    # ---------------- phase A: transpose hidden states on PE ----------------
    dma_engines = [nc.gpsimd, nc.gpsimd, nc.gpsimd, nc.gpsimd]

    def emit_hs_chunk(rc):
        nats = []
        for j in range(4):
            st = rc * 4 + j
            src, r0 = (hs_q, st * 128) if st < 8 else (hs_o, (st - 8) * 128)
            nat = tr_pool.tile([128, H], BF16, name="nat", tag="nat")
            dma_engines[j].dma_start(out=nat, in_=src[r0:r0 + 128, :])
            nats.append(nat)
        for ht in range(HT):
            ps = mm_ps.tile([128, 512], BF16, name="mmt", tag="mm")
            for j, nat in enumerate(nats):
                nc.tensor.transpose(ps[:, j * 128:(j + 1) * 128],
                                    nat[:, ht * 128:(ht + 1) * 128], ident)
            nc.vector.tensor_copy(hsT8[:, ht, rc * 512:(rc + 1) * 512], ps)

    def proj_kq(hp, wk8, wq8):
        for sc in range(S // 512):
            ps = mm_ps.tile([128, 512], F32, name="mm", tag="mm")
            for j in range(HT // 2):
                nc.tensor.matmul(ps, wk8[:, 2 * j:2 * j + 2, hp * 128:(hp + 1) * 128],
                                 hsT8[:, 2 * j:2 * j + 2, sc * 512:(sc + 1) * 512],
                                 start=(j == 0), stop=(j == HT // 2 - 1),
                                 perf_mode=DR)
            nc.vector.tensor_scalar(out=kT[hp][:, sc * 512:(sc + 1) * 512], in0=ps,
                                    scalar1=1.0 / W_SCALE,
                                    scalar2=bkc[:, hp:hp + 1],
                                    op0=OP.mult, op1=OP.add)
        for qc in range(QB):
            ps = mm_ps.tile([128, 512], F32, name="mm", tag="mm")
            for j in range(HT // 2):
                nc.tensor.matmul(ps, wq8[:, 2 * j:2 * j + 2, hp * 128:(hp + 1) * 128],
                                 hsT8[:, 2 * j:2 * j + 2, qc * 512:(qc + 1) * 512],
                                 start=(j == 0), stop=(j == HT // 2 - 1),
                                 perf_mode=DR)
            nc.vector.tensor_scalar(out=qT[hp][:, qc * 512:(qc + 1) * 512], in0=ps,
                                    scalar1=(A_SCHR / 8.0) / W_SCALE,
                                    scalar2=bqc[:, hp:hp + 1],
                                    op0=OP.mult, op1=OP.add)

    def proj_v(st, wv8):
        """V rows for key-tile st -> vS bf16 strided head layout + ones col."""
        vv = vS[st].rearrange("p (h e) -> p h e", e=HD + 1)
        for dc in range(2):
            ps = mm_ps.tile([128, 512], F32, name="mm", tag="mm")
            for j in range(HT // 2):
                nc.tensor.matmul(ps, hsT8[:, 2 * j:2 * j + 2, st * 128:(st + 1) * 128],
                                 wv8[:, 2 * j:2 * j + 2, dc * 512:(dc + 1) * 512],
                                 start=(j == 0), stop=(j == HT // 2 - 1),
                                 perf_mode=DR)
            if zb:
                nc.vector.tensor_scalar(
                    out=vv[:, dc * 8:(dc + 1) * 8, 0:HD],
                    in0=ps.rearrange("p (h e) -> p h e", e=HD),
                    scalar1=1.0 / W_SCALE, scalar2=None, op0=OP.mult)
            else:
                sb = nrm_pool.tile([128, 512], BF16, name="vsb", tag="vsb")
                nc.vector.tensor_scalar(out=sb, in0=ps, scalar1=1.0 / W_SCALE,
                                        scalar2=None, op0=OP.mult)
                nc.vector.tensor_tensor(
                    out=vv[:, dc * 8:(dc + 1) * 8, 0:HD],
                    in0=sb.rearrange("p (h e) -> p h e", e=HD),
                    in1=bvb[:, dc * 512:(dc + 1) * 512].rearrange(
                        "p (h e) -> p h e", e=HD),
                    op=OP.add)
        nc.vector.memset(vv[:, :, HD:HD + 1], 1.0)

    def attn_head(h, finish_prev=None):
        """Emit head h; returns a closure finishing its normalize (reciprocal
        + broadcast + cT8 writes). finish_prev is invoked mid-loop so the
        previous head's broadcast DMA round-trip hides under this head's
        compute instead of head-of-line-blocking the DVE queue."""
        hp, hh = divmod(h, 2)
        drows = slice(hh * 64, hh * 64 + 64)
        ctx_ps = [cx_ps.tile([HD + 1, 512], F32, name="cx", tag=f"cx{qc}")
                  for qc in range(QB)]
        for kt in range(ST):
            sps = sc_ps.tile([128, SH], F32, name="sc", tag="sc")
            for qc in range(QB):
                nc.tensor.matmul(sps[:, qc * 512:(qc + 1) * 512],
                                 kT[hp][drows, kt * 128:(kt + 1) * 128],
                                 qT[hp][drows, qc * 512:(qc + 1) * 512],
                                 start=True, stop=True)
            pt = p_pool.tile([128, SH], BF16, name="pt", tag="pt")
            if N_DVE_KT and kt % (ST // N_DVE_KT) == 1:
                nc.vector.tensor_scalar(out=pt.bitcast(I16), in0=sps,
                                        scalar1=16.0, scalar2=C16,
                                        op0=OP.mult, op1=OP.add)
            else:
                nc.scalar.activation(pt, sps, AF.Exp,
                                     bias=nl8_t, scale=1.0 / A_SCHR)
            for qc in range(QB):
                nc.tensor.matmul(ctx_ps[qc],
                                 vS[kt][:, h * (HD + 1):(h + 1) * (HD + 1)],
                                 pt[:, qc * 512:(qc + 1) * 512],
                                 start=(kt == 0), stop=(kt == ST - 1))
            if kt == 5 and finish_prev is not None:
                finish_prev()
        # stage out of PSUM (frees ctx accumulators) and gather the sums rows
        stage = [nrm_pool.tile([HD + 1, 512], F32, name="stage", tag=f"st{qc}")
                 for qc in range(QB)]
        for qc in range(QB):
            nc.vector.tensor_copy(stage[qc], ctx_ps[qc])
        srow = dram_pool.tile([1, SH], F32, name="srow", tag="srow", bufs=2)
        for qc in range(QB):
            nc.sync.dma_start(out=srow[:, qc * 512:(qc + 1) * 512],
                              in_=stage[qc][HD:HD + 1, :])
        sg = nrm_pool.tile([8, 128], F32, name="sg", tag="sg")
        nc.sync.dma_start(out=sg, in_=srow.rearrange("o (p f) -> p (o f)", p=8))

        def finish():
            nc.vector.tensor_scalar(out=sg, in0=sg, scalar1=0.25, scalar2=None,
                                    op0=OP.mult)
            nc.vector.reciprocal(sg, sg)
            rrow = dram_pool.tile([1, SH], F32, name="rrow", tag="rrow", bufs=2)
            nc.sync.dma_start(out=rrow.rearrange("o (p f) -> p (o f)", p=8),
                              in_=sg)
            recb = nrm_pool.tile([HD, SH], F32, name="recb", tag="recb")
            nc.sync.dma_start(out=recb, in_=rrow.partition_broadcast(HD))
            for qc in range(QB):
                nc.vector.tensor_tensor(
                    out=cT8[drows, hp, qc * 512:(qc + 1) * 512],
                    in0=stage[qc][0:HD, :],
                    in1=recb[:, qc * 512:(qc + 1) * 512], op=OP.mult)

        return finish

    # ---------------- emission --------------------------------------------
    wv_pool = tc.alloc_tile_pool(name="wvpool", bufs=1)
    wv8 = load_w8(wv_pool, wv8_d, "wv8")
    wk8 = load_w8(wkq_pool, wk8_d, "wk8")
    wq8 = load_w8(wkq_pool, wq8_d, "wq8")
    for rc in range(4):
        emit_hs_chunk(rc)
        for st in range(rc * 4, rc * 4 + 4):
            proj_v(st, wv8)
    wv_pool.release()
    tr_pool.release()

    for hp in range(HP - 1):
        proj_kq(hp, wk8, wq8)
        attn_head(2 * hp)()
        attn_head(2 * hp + 1)()
    proj_kq(HP - 1, wk8, wq8)
    wkq_pool.release()

    # phase-D pools open early: wo + LN constants stream during last heads
    wo_pool = tc.alloc_tile_pool(name="wopool", bufs=1)
    wo8 = load_w8(wo_pool, wo8_d, "wo8")
    d_pool = tc.alloc_tile_pool(name="dpool", bufs=3)
    dc_pool = tc.alloc_tile_pool(name="dcpool", bufs=1)
    if not zb:
        bob = dc_pool.tile([128, H], F32, name="bob")
        nc.gpsimd.dma_start(out=bob,
                            in_=bo_d.rearrange("(o n) -> o n", o=1).partition_broadcast(128))
    if not ln_id:
        gam_b = dc_pool.tile([128, H], F32, name="gam_b")
        nc.sync.dma_start(out=gam_b,
                          in_=gam_d.rearrange("(o n) -> o n", o=1).partition_broadcast(128))
        bet_b = dc_pool.tile([128, H], F32, name="bet_b")
        nc.sync.dma_start(out=bet_b,
                          in_=bet_d.rearrange("(o n) -> o n", o=1).partition_broadcast(128))

    attn_head(NH - 2)()
    attn_head(NH - 1)()

    # ---------------- phase D: out proj + residual + LayerNorm -------------
    hs_rows = hs_q.rearrange("(t p) n -> t p n", p=128)
    out_rows = out_d.rearrange("(t p) n -> t p n", p=128)
    for blk in range(SH // 128):
        res = d_pool.tile([128, H], F32, name="res", tag="res")
        nc.sync.dma_start(out=res, in_=hs_rows[blk])
        nc.vector.tensor_scalar(out=res, in0=res, scalar1=64.0,
                                scalar2=None, op0=OP.mult)
        if not zb:
            nc.vector.tensor_tensor(out=res, in0=res, in1=bob, op=OP.add)
        x = d_pool.tile([128, H], F32, name="x", tag="x")
        for ec in range(2):
            ps = mm_ps.tile([128, 512], F32, name="mm", tag="mm")
            for j in range(HT // 2):
                nc.tensor.matmul(ps, cT8[:, 2 * j:2 * j + 2, blk * 128:(blk + 1) * 128],
                                 wo8[:, 2 * j:2 * j + 2, ec * 512:(ec + 1) * 512],
                                 start=(j == 0), stop=(j == HT // 2 - 1),
                                 perf_mode=DR)
            nc.vector.tensor_tensor(out=x[:, ec * 512:(ec + 1) * 512],
                                    in0=ps, in1=res[:, ec * 512:(ec + 1) * 512],
                                    op=OP.add)
        stats = d_pool.tile([128, 2, 6], F32, name="stats", tag="stats")
        xg = x.rearrange("p (g n) -> p g n", g=2)
        for g in range(2):
            nc.vector.bn_stats(out=stats[:, g, :], in_=xg[:, g, :])
        mv = d_pool.tile([128, 2], F32, name="mv", tag="mv")
        nc.vector.bn_aggr(out=mv, in_=stats)
        rstd = d_pool.tile([128, 1], F32, name="rstd", tag="rstd")
        nc.scalar.activation(rstd, mv[:, 1:2], AF.Sqrt, bias=eps_t)
        nc.vector.reciprocal(rstd, rstd)
        nmu = d_pool.tile([128, 1], F32, name="nmu", tag="nmu")
        nc.vector.tensor_tensor(out=nmu, in0=mv[:, 0:1], in1=rstd, op=OP.mult)
        nc.vector.tensor_scalar_mul(nmu, nmu, -1.0)
        y = d_pool.tile([128, H], F32, name="y", tag="y")
        nc.vector.tensor_scalar(out=y, in0=x, scalar1=rstd, scalar2=nmu,
                                op0=OP.mult, op1=OP.add)
        if not ln_id:
            nc.vector.tensor_tensor(out=y, in0=y, in1=gam_b, op=OP.mult)
            nc.vector.tensor_tensor(out=y, in0=y, in1=bet_b, op=OP.add)
        nc.sync.dma_start(out=out_rows[blk], in_=y)

    for pool in (dc_pool, d_pool, wo_pool, p_pool, nrm_pool, dram_pool,
                 cx_ps, sc_ps, mm_ps, const_p, persist):
        pool.release()


def build_nc(ln_id=True, zb=True):
    key = (ln_id, zb)
    if key in _CACHED_NC:
        return _CACHED_NC[key]
    nc = bacc.Bacc("TRN2", target_bir_lowering=False, debug=False,
                   num_devices=N_CORES)
    with tile.TileContext(nc) as tc:
        _emit(tc, ln_id, zb)
    nc.compile()
    _CACHED_NC[key] = nc
    return nc


def make_in_maps(inputs):
    import ml_dtypes
    hs = np.ascontiguousarray(np.asarray(inputs["hidden_states"], dtype=np.float32))
    w8 = {k: np.ascontiguousarray(
              (np.asarray(inputs[k], np.float32).T * W_SCALE)
              .astype(ml_dtypes.float8_e4m3))
          for k in ("Wq", "Wk", "Wv", "Wo")}
    com = {
        "wq8": w8["Wq"], "wk8": w8["Wk"], "wv8": w8["Wv"], "wo8": w8["Wo"],
        "bq": np.asarray(inputs["bq"], np.float32) * (A_SCHR / 8.0),
        "bk": np.asarray(inputs["bk"], np.float32),
        "bv": np.asarray(inputs["bv"], np.float32).astype(ml_dtypes.bfloat16),
        "bo": np.asarray(inputs["bo"], np.float32) * 64.0,
        "ln_gamma": np.asarray(inputs["ln_gamma"], np.float32),
        "ln_beta": np.asarray(inputs["ln_beta"], np.float32),
    }
    in_maps = []
    for c in range(N_CORES):
        b, sb = divmod(c, 2)
        in_maps.append({
            "hs_q": np.ascontiguousarray(hs[b, sb * SH:(sb + 1) * SH]),
            "hs_o": np.ascontiguousarray(hs[b, (1 - sb) * SH:(2 - sb) * SH]),
            **com,
        })
    return in_maps


def gather_out(results):
    out = np.empty((B, S, H), np.float32)
    for c in range(N_CORES):
        b, sb = divmod(c, 2)
        out[b, sb * SH:(sb + 1) * SH, :] = results[c]["out"]
    return out


def kernel(**inputs) -> np.ndarray:
    ln_id = (np.all(np.asarray(inputs["ln_gamma"]) == 1.0)
             and np.all(np.asarray(inputs["ln_beta"]) == 0.0))
    zb = all(np.all(np.asarray(inputs[k]) == 0.0)
             for k in ("bq", "bk", "bv", "bo"))
    nc = build_nc(bool(ln_id), bool(zb))
    res = run_bass_kernel_spmd(nc, make_in_maps(inputs), list(range(N_CORES)))
    return gather_out(res.results)
